# revision 20
# baseline (speedup 1.0000x reference)
"""LlamaAttention (B=1, S=2048, D=2048, H=16, KV=4) on 8 TRN2 NeuronCores.

Tensor-parallel over heads: core c owns q-heads [2c, 2c+1] and kv-head c//2.
Each core computes partial = attn_out_c @ Wo[:, c-slice].T over the full
sequence; the all-reduce after o_proj happens on the host (sum of partials).

Layout strategy: everything on-chip lives feature-on-partitions ("transposed"):
  hsT [d, s], qT/kT/vT [hd, s], attn_outT [hd, s].  The host pre-transposes
hidden_states and weights into partition-major [128, N] bf16 arrays so every
DMA is contiguous; rope tables (bf16 cos / sign-adjusted sin) and the causal
diagonal mask block are precomputed on host.

Schedule (all matmuls bf16: fast weight load, half HBM):
 - DMA prologue interleaves weight chunks with hs tiles in consumption order
   so quarter-0 QKV matmuls never stall on a bulk weight transfer.
 - QKV projects in four 512-column PSUM quarters (4 banks), leaving 4 banks
   for attention score chunks: units 0-5's score/softmax chunks interleave
   into the back half of the projection matmul stream.
 - Late softmax normalization: P is left UNNORMALIZED in SBUF; its xbar
   transpose is issued immediately after the tile's last exp chunk (no
   dependency on the row sums), so the transpose channel starts as early as
   possible.  The row sums l are transposed on the PE (tiny fp32 matmul
   against an identity) and inverted once per unit; P@V psum chunks are
   drained with a fused multiply by the broadcast 1/l row, which also casts
   to bf16.
 - Remaining units' score generation is pumped one chunk at a time between
   P@V accumulation steps, and each group's o_proj tiles are split around
   unit seams, so the in-order PE queue always has matmul work while
   exp (scalar) and normalization (vector) catch up.
 - softmax: exp with accum_out row sums (no running max: scores are O(6)
   sigma so fp32 exp cannot overflow).
 - PSUM drains: rope uses one scalar cast then all-bf16 DVE ops (the
   rotate-half is a partition-shifted DVE copy); o_proj casts alternate
   scalar/vector.  Output partials are bf16, host all-reduces in f32.
"""
import math
import numpy as np

S = 2048
D = 2048
HD = 128
H = 16
KV = 4
NCORES = 8
NT = S // 128          # 16 sequence tiles
DTC = D // 128         # 16 feature chunks
QH = H // NCORES       # 2 q-heads per core
ROPE_BASE = 10000.0
SCALE = 1.0 / math.sqrt(HD)
NEG = -1.0e9

_CACHE = {}


def _rope(nc, pool, dst, src_ps, cos_sb, sin_sb, cols, BF16, ALU):
    """dst[:, cols] = src*cos + rotate_half(src)*sin  (src: psum [128, w]).

    One scalar drain (psum->bf16 sbuf), then all-bf16 SBUF vector ops which
    run in the DVE's fast 2x/4x modes; the rotate-half is a partition-shifted
    DVE copy (legal on TRN2).
    """
    w = cols.stop - cols.start
    raw = pool.tile([128, w], BF16, tag="roperaw")
    rot = pool.tile([128, w], BF16, tag="roperot")
    t1 = pool.tile([128, w], BF16, tag="ropet1")
    nc.scalar.copy(out=raw, in_=src_ps)
    nc.vector.tensor_copy(out=rot[0:64, :], in_=raw[64:128, :])
    nc.vector.tensor_copy(out=rot[64:128, :], in_=raw[0:64, :])
    nc.vector.tensor_tensor(out=t1, in0=raw, in1=cos_sb[:, cols], op=ALU.mult)
    nc.vector.tensor_tensor(out=rot, in0=rot, in1=sin_sb[:, cols], op=ALU.mult)
    nc.vector.tensor_tensor(out=dst[:, cols], in0=t1, in1=rot, op=ALU.add)


def build_nc():
    import concourse.bacc as bacc
    import concourse.tile as tile
    from concourse import mybir

    F32 = mybir.dt.float32
    BF16 = mybir.dt.bfloat16
    AF = mybir.ActivationFunctionType
    ALU = mybir.AluOpType

    nc = bacc.Bacc("TRN2", target_bir_lowering=False, debug=False)
    hs_d = nc.dram_tensor("hs", [128, DTC * S], BF16, kind="ExternalInput").ap()
    wq_d = nc.dram_tensor("wq", [128, DTC * QH * 128], BF16, kind="ExternalInput").ap()
    wk_d = nc.dram_tensor("wk", [128, DTC * 128], BF16, kind="ExternalInput").ap()
    wv_d = nc.dram_tensor("wv", [128, DTC * 128], BF16, kind="ExternalInput").ap()
    wo_d = nc.dram_tensor("wo", [128, QH * D], BF16, kind="ExternalInput").ap()
    cos_d = nc.dram_tensor("cos", [128, S], BF16, kind="ExternalInput").ap()
    sin_d = nc.dram_tensor("sin", [128, S], BF16, kind="ExternalInput").ap()
    tri_d = nc.dram_tensor("tri", [128, 128], F32, kind="ExternalInput").ap()
    eye_d = nc.dram_tensor("eye", [128, 128], BF16, kind="ExternalInput").ap()
    out_d = nc.dram_tensor("out", [128, NT * D], BF16, kind="ExternalOutput").ap()

    hs3 = hs_d.rearrange("p (t s) -> p t s", t=DTC)
    out3 = out_d.rearrange("p (t d) -> p t d", t=NT)

    HALF = S // 2
    QTR = S // 4

    with tile.TileContext(nc) as tc:
        with tc.tile_pool(name="consts", bufs=1) as consts, \
             tc.tile_pool(name="persist", bufs=1) as persist, \
             tc.tile_pool(name="stats", bufs=1) as stats, \
             tc.tile_pool(name="pp", bufs=8) as pp, \
             tc.tile_pool(name="ptt", bufs=3) as ptt, \
             tc.tile_pool(name="osb", bufs=4) as osb, \
             tc.tile_pool(name="lbcp", bufs=4) as lbcp, \
             tc.tile_pool(name="sps", bufs=2, space="PSUM") as sps:
            tri_sb = consts.tile([128, 128], F32)
            eye_sb = consts.tile([128, 128], BF16)
            cos_sb = consts.tile([128, S], BF16)
            sin_sb = consts.tile([128, S], BF16)
            wq_sb = consts.tile([128, DTC, QH * 128], BF16)
            wk_sb = consts.tile([128, DTC, 128], BF16)
            wv_sb = consts.tile([128, DTC, 128], BF16)
            wo_sb = consts.tile([128, QH, D], BF16)

            qrot = [persist.tile([128, S], BF16, tag=f"qrot{h}", name=f"qrot{h}") for h in range(QH)]
            krot = persist.tile([128, S], BF16, tag="krot")
            vbf = persist.tile([128, S], BF16, tag="vbf")
            vnat = persist.tile([128, NT * 128], BF16, tag="vnat")
            vnat3 = vnat.rearrange("p (t f) -> p t f", t=NT)
            aout = [persist.tile([128, S], BF16, tag=f"aout{h}", name=f"aout{h}") for h in range(QH)]
            l_sb = stats.tile([128, QH * NT], F32, tag="l")
            lpart = stats.tile([128, QH * NT * 2], F32, tag="lpart")
            linv_sb = stats.tile([128, QH * NT], F32, tag="linv")
            linv_bf = stats.tile([128, QH * NT], BF16, tag="linvbf")

            units = [(g, h) for g in range(NT // 4) for h in range(QH)]

            def scores_gen(u):
                """QK chunks + mask + exp + eager P^T xbar for unit u.

                P is left unnormalized; each i-tile's transpose is issued
                right after its last exp chunk.  After the last tile, l for
                the unit's 4 tiles is transposed on the PE (into a corner of
                the last scores psum chunk), inverted on the DVE, and
                replicated across partitions on the idle GpSimd engine.
                Yields after each score chunk so the caller can interleave
                other engine work; returns (ptall, lbc) via
                StopIteration.value.
                """
                g, h = units[u]
                c0u = h * NT + 4 * g
                for ii in range(4):
                    i = 4 * g + ii
                    W = (i + 1) * 128
                    p_i = pp.tile([128, S], BF16, tag="p", name=f"p{u}_{ii}")
                    col = h * NT + i
                    nch = (W + 1023) // 1024
                    for c in range(nch):
                        c0 = 1024 * c
                        ce = min(c0 + 1024, W)
                        s_ch = sps.tile([128, 1024], F32, tag="s")
                        for m0 in range(c0, ce, 512):
                            m1 = min(m0 + 512, ce)
                            nc.tensor.matmul(s_ch[:, m0 - c0:m1 - c0],
                                             qrot[h][:, i * 128:(i + 1) * 128],
                                             krot[:, m0:m1], start=True, stop=True)
                        if ce == W:   # diagonal block lives in this chunk
                            nc.vector.tensor_tensor(
                                out=s_ch[:, W - 128 - c0:W - c0],
                                in0=s_ch[:, W - 128 - c0:W - c0],
                                in1=tri_sb, op=ALU.add)
                        acc = (l_sb[:, col:col + 1] if nch == 1
                               else lpart[:, col * 2 + c:col * 2 + c + 1])
                        nc.scalar.activation(out=p_i[:, c0:ce], in_=s_ch[:, 0:ce - c0],
                                             func=AF.Exp, scale=SCALE,
                                             accum_out=acc)
                        yield
                    if nch > 1:
                        nc.vector.tensor_reduce(out=l_sb[:, col:col + 1],
                                                in_=lpart[:, col * 2:col * 2 + nch],
                                                axis=mybir.AxisListType.X, op=ALU.add)
                    if ii == 0:
                        ptall = ptt.tile([128, 4, NT, 128], BF16, tag="ptall",
                                         name=f"ptall{u}")
                    nc.sync.dma_start_transpose(
                        out=ptall[:, ii, 0:W // 128, :], in_=p_i[:, 0:W])
                # Invert l in the fast 128-lane orientation, cast to bf16, and
                # transpose onto partition 0 of the unit's last scores psum
                # chunk with tiny bf16 matmuls; a scalar copy lands the 1/l
                # row in SBUF for the GpSimd partition broadcasts.
                lbc = lbcp.tile([128, 512], F32, tag="lbc", name=f"lbc{u}")
                nc.vector.reciprocal(out=linv_sb[:, c0u:c0u + 4],
                                     in_=l_sb[:, c0u:c0u + 4])
                nc.vector.tensor_copy(out=linv_bf[:, c0u:c0u + 4],
                                      in_=linv_sb[:, c0u:c0u + 4])
                for ii in range(4):
                    nc.tensor.matmul(s_ch[0:1, ii * 128:(ii + 1) * 128],
                                     linv_bf[:, c0u + ii:c0u + ii + 1],
                                     eye_sb, start=True, stop=True)
                nc.scalar.copy(out=lbc[0:1, 0:512], in_=s_ch[0:1, 0:512])
                for ii in range(4):
                    nc.gpsimd.partition_broadcast(
                        lbc[:, ii * 128:(ii + 1) * 128],
                        lbc[0:1, ii * 128:(ii + 1) * 128])
                return ptall, lbc

            def run_gen(gen):
                while True:
                    try:
                        next(gen)
                    except StopIteration as e:
                        return e.value

            # ---------------- QKV in quarters + early attention ------------
            hst_tiles = {}

            with tc.tile_pool(name="hsp", bufs=8) as hsp, \
                 tc.tile_pool(name="ropet", bufs=1) as ropet, \
                 tc.tile_pool(name="qkvps", bufs=1, space="PSUM") as qkvps:

                def load_hst(sh, j):
                    t = hsp.tile([128, 2, HALF], BF16, tag="hst", name=f"hst{sh}_{j}")
                    nc.sync.dma_start(
                        out=t, in_=hs3[:, 2 * j:2 * j + 2, sh * HALF:(sh + 1) * HALF])
                    hst_tiles[(sh, j)] = t
                    return t

                # DMA prologue on two HWDGE rings: hs tiles on the sync ring,
                # weight chunks (need-ordered) + rope tables on the scalar
                # ring, so neither queue's per-issue cost (~0.7us) serializes
                # against the other and quarter-0 never waits on one bulk
                # transfer.
                wq3 = wq_d.rearrange("p (t m) -> p t m", t=DTC)
                wk3 = wk_d.rearrange("p (t m) -> p t m", t=DTC)
                wv3 = wv_d.rearrange("p (t m) -> p t m", t=DTC)

                def load_w(a, b):
                    nc.scalar.dma_start(out=wq_sb[:, a:b, :], in_=wq3[:, a:b, :])
                    nc.scalar.dma_start(out=wk_sb[:, a:b, :], in_=wk3[:, a:b, :])
                    nc.scalar.dma_start(out=wv_sb[:, a:b, :], in_=wv3[:, a:b, :])

                nc.sync.dma_start(out=tri_sb, in_=tri_d)
                load_w(0, 2)
                load_hst(0, 0)
                load_w(2, 6)
                load_hst(0, 1)
                load_w(6, 11)
                load_hst(0, 2)
                load_w(11, 16)
                for j in range(3, 8):
                    load_hst(0, j)
                nc.scalar.dma_start(out=cos_sb, in_=cos_d)
                nc.scalar.dma_start(out=sin_sb, in_=sin_d)
                nc.scalar.dma_start(out=eye_sb, in_=eye_d)
                nc.scalar.dma_start(out=wo_sb, in_=wo_d.rearrange("p (h m) -> p h m", h=QH))

                early = []
                pending = []
                for qtr in range(4):
                    sh, qq = divmod(qtr, 2)
                    cols = slice(qtr * QTR, (qtr + 1) * QTR)
                    if qtr == 1:
                        for j in range(8):   # prefetch half 1 as slots free up
                            load_hst(1, j)
                    if qtr == 2:
                        pending = [scores_gen(0), scores_gen(1)]
                    if qtr == 3:
                        pending += [scores_gen(2), scores_gen(3), scores_gen(4),
                                    scores_gen(5)]
                    pq = [qkvps.tile([128, QTR], F32, tag=f"pq{m}", name=f"pq{m}") for m in range(QH)]
                    pk = qkvps.tile([128, QTR], F32, tag="pk")
                    pv = qkvps.tile([128, QTR], F32, tag="pv")
                    for j in range(DTC // 2):
                        hst = hst_tiles[(sh, j)]
                        for t2 in range(2):
                            dt = 2 * j + t2
                            st = dt == 0
                            sp = dt == DTC - 1
                            wlist = ([(wq_sb[:, dt, m * 128:(m + 1) * 128], pq[m]) for m in range(QH)]
                                     + [(wk_sb[:, dt, :], pk), (wv_sb[:, dt, :], pv)])
                            for w_ap, dst in wlist:
                                nc.tensor.matmul(dst, w_ap, hst[:, t2, qq * QTR:(qq + 1) * QTR],
                                                 start=st, stop=sp)
                        for _ in range(2 if qtr == 3 else 1):
                            if pending:
                                try:
                                    next(pending[0])
                                except StopIteration as e:
                                    early.append(e.value)
                                    pending.pop(0)
                    for m in range(QH):
                        _rope(nc, ropet, qrot[m], pq[m], cos_sb, sin_sb, cols, BF16, ALU)
                    _rope(nc, ropet, krot, pk, cos_sb, sin_sb, cols, BF16, ALU)
                    nc.scalar.copy(out=vbf[:, cols], in_=pv)
                    nc.sync.dma_start_transpose(
                        out=vnat3[:, 4 * qtr:4 * qtr + 4, :], in_=vbf[:, cols])


                while pending:
                    early.append(run_gen(pending.pop(0)))

            # ---------------- attention tail + fused o_proj ----------------
            with tc.tile_pool(name="pvps", bufs=2, space="PSUM") as pvps, \
                 tc.tile_pool(name="pops", bufs=2, space="PSUM") as pops:

                deferred = []

                def oproj_tile(t, g):
                    o_sb = osb.tile([128, D], BF16, tag="osb")
                    for n in range(D // 512):
                        po = pops.tile([128, 512], F32, tag="po", name=f"po{t}_{n}")
                        for hh in range(QH):
                            nc.tensor.matmul(po, aout[hh][:, t * 128:(t + 1) * 128],
                                             wo_sb[:, hh, n * 512:(n + 1) * 512],
                                             start=(hh == 0), stop=(hh == QH - 1))
                        on_scalar = (n % 2 == 0) if g == 3 else (n == 0)
                        if on_scalar:
                            nc.scalar.copy(out=o_sb[:, n * 512:(n + 1) * 512], in_=po)
                        else:
                            nc.vector.tensor_copy(out=o_sb[:, n * 512:(n + 1) * 512], in_=po)
                    # store on the idle GpSimd SWDGE so the sync ring stays
                    # free for the P^T transposes
                    nc.gpsimd.dma_start(out=out3[:, t, :], in_=o_sb)

                def stage_pv(u, pt_lbc, pump):
                    """P@V accumulation + normalized aout drain; o_proj split
                    around unit seams."""
                    g, h = units[u]
                    ptall, lbc = pt_lbc
                    jmax = 4 * g + 3
                    pv_ps = pvps.tile([128, 512], F32, tag="pv")
                    for j in range(jmax + 1):
                        ii_lo = max(0, j - 4 * g)
                        nc.tensor.matmul(pv_ps[:, ii_lo * 128:512],
                                         vnat[:, j * 128:(j + 1) * 128],
                                         ptall[:, ii_lo:4, j, :],
                                         start=(j == 0), stop=(j == jmax))
                        pump()
                        if deferred and j % 3 == 2:   # PE filler between PV steps
                            oproj_tile(*deferred.pop(0))
                    while deferred:
                        oproj_tile(*deferred.pop(0))
                    # fused drain: aout = pv * (1/l rows), one 512-col DVE op
                    nc.vector.tensor_tensor(
                        out=aout[h][:, 4 * g * 128:(4 * g + 4) * 128],
                        in0=pv_ps, in1=lbc, op=ALU.mult)
                    if h == QH - 1:   # both heads done: 1 tile now, 3 deferred
                        oproj_tile(4 * g, g)
                        deferred.extend([(4 * g + 1, g), (4 * g + 2, g),
                                         (4 * g + 3, g)])

                # Pump the next units' score generation between PV matmuls so
                # the in-order PE queue always has matmul work while softmax
                # (scalar/vector) of later units catches up.
                ptalls = dict(enumerate(early))
                live = {u: scores_gen(u) for u in range(len(early), len(units))}

                rr = [0]

                def pump():
                    # round-robin across live units so a later unit's exps and
                    # transposes interleave instead of all landing at the tail
                    keys = sorted(live)
                    if not keys:
                        return
                    u = keys[rr[0] % len(keys)]
                    rr[0] += 1
                    try:
                        next(live[u])
                    except StopIteration as e:
                        ptalls[u] = e.value
                        del live[u]

                for u in range(len(units)):
                    while u not in ptalls:   # finish its scores if still pending
                        pump()
                    stage_pv(u, ptalls.pop(u), pump)
                while deferred:
                    oproj_tile(*deferred.pop(0))

    nc.compile()
    return nc


def _pm(x):
    """[n*128, M] row-major -> partition-major [128, n*M]."""
    n = x.shape[0] // 128
    return np.ascontiguousarray(
        x.reshape(n, 128, x.shape[1]).transpose(1, 0, 2).reshape(128, -1))


def prep_in_maps(hidden_states, position_ids, Wq, Wk, Wv, Wo):
    import ml_dtypes
    BF = ml_dtypes.bfloat16
    hs = np.asarray(hidden_states, dtype=np.float32).reshape(S, D)
    hsT_pm = _pm(np.ascontiguousarray(hs.T)).astype(BF)             # [128, DTC*S]

    pos = np.asarray(position_ids).reshape(S).astype(np.float32)
    inv = (ROPE_BASE ** (-np.arange(0, HD, 2, dtype=np.float32) / HD))  # [64]
    ang = np.concatenate([pos[None, :] * inv[:, None]] * 2, axis=0)     # [128, S]
    cos_t = np.cos(ang).astype(BF)
    sin_t = np.sin(ang).astype(np.float32)
    sin_signed = np.concatenate([-sin_t[:64], sin_t[64:]], axis=0).astype(BF)

    q_idx = np.arange(128)[:, None]
    k_idx = np.arange(128)[None, :]
    tri = np.where(k_idx <= q_idx, 0.0, NEG).astype(np.float32)
    eye = np.eye(128, dtype=np.float32).astype(BF)

    Wq = np.asarray(Wq, np.float32)
    Wk = np.asarray(Wk, np.float32)
    Wv = np.asarray(Wv, np.float32)
    Wo = np.asarray(Wo, np.float32)

    in_maps = []
    for c in range(NCORES):
        g = (c * QH) // (H // KV)          # kv head owned by this core
        wq_c = Wq[c * QH * 128:(c + 1) * QH * 128]      # [256, D]
        wk_c = Wk[g * 128:(g + 1) * 128]                # [128, D]
        wv_c = Wv[g * 128:(g + 1) * 128]                # [128, D]
        wo_c = Wo[:, c * QH * 128:(c + 1) * QH * 128]   # [D, 256]
        in_maps.append({
            "hs": hsT_pm,
            "wq": _pm(np.ascontiguousarray(wq_c.T)).astype(BF),
            "wk": _pm(np.ascontiguousarray(wk_c.T)).astype(BF),
            "wv": _pm(np.ascontiguousarray(wv_c.T)).astype(BF),
            "wo": _pm(np.ascontiguousarray(wo_c.T)).astype(BF),
            "cos": cos_t,
            "sin": sin_signed,
            "tri": tri,
            "eye": eye,
        })
    return in_maps


def combine_outputs(results):
    total = np.zeros((S, D), np.float32)
    for r in results:
        o = np.asarray(r["out"], np.float32)
        total += o.reshape(128, NT, D).transpose(1, 0, 2).reshape(S, D)
    return total[None]


def kernel(hidden_states, attention_mask, position_ids, Wq, Wk, Wv, Wo):
    from concourse import bass_utils
    if "nc" not in _CACHE:
        _CACHE["nc"] = build_nc()
    nc = _CACHE["nc"]
    in_maps = prep_in_maps(hidden_states, position_ids, Wq, Wk, Wv, Wo)
    res = bass_utils.run_bass_kernel_spmd(nc, in_maps, core_ids=list(range(NCORES)))
    return combine_outputs(res.results)


# revision 22
# speedup vs baseline: 1.0078x; 1.0078x over previous
"""LlamaAttention (B=1, S=2048, D=2048, H=16, KV=4) on 8 TRN2 NeuronCores.

Tensor-parallel over heads: core c owns q-heads [2c, 2c+1] and kv-head c//2.
Each core computes partial = attn_out_c @ Wo[:, c-slice].T over the full
sequence; the all-reduce after o_proj happens on the host (sum of partials).

Layout strategy: everything on-chip lives feature-on-partitions ("transposed"):
  hsT [d, s], qT/kT/vT [hd, s], attn_outT [hd, s].  The host pre-transposes
hidden_states and weights into partition-major [128, N] bf16 arrays so every
DMA is contiguous; rope tables (bf16 cos / sign-adjusted sin) and the causal
diagonal mask block (transposed) are precomputed on host.

Key design point vs a classic flash-style schedule: attention scores are
computed TRANSPOSED (s^T[k, q] via stationary = krot k-tile, moving = qrot
q-columns), so the exp writes P^T [k-on-partitions] straight into SBUF and
P@V consumes it directly as the moving operand.  No DMA xbar transposes of P
are needed (each DMA transpose acts as a full DMA-subsystem barrier on TRN2,
which serialized the previous design).  The softmax row sums l[q] are
accumulated with an all-ones stationary matmul into a PSUM tile alongside
the P@V accumulation (every output partition holds the same l row), so the
aout drain is one reciprocal + one multiply, with no cross-partition
broadcast.

Schedule (all matmuls bf16):
 - QKV projects per quarter in TWO 2-bank passes (q-heads, then k/v), so
   PSUM fits: 2 qkv + 2 scores + 1 l + 1 pv + 2 o_proj = 8 banks.
 - attention chunk c (scores c for all k-tiles j<=4c+3, l+PV for units
   g=c, o_proj group c) interleaves into quarter c+1's projection stream;
   chunk 3 forms the tail as a per-j software pipeline
   [score MM j+1 | exp j | l/PV MM j] so the PE never waits a full exp.
 - softmax: plain exp (no running max: scores are O(6) sigma so fp32 exp
   cannot overflow); P^T stays unnormalized bf16, aout = pv * (1/l).
 - PSUM drains: rope uses one scalar cast then all-bf16 DVE ops (the
   rotate-half is a partition-shifted DVE copy); o_proj casts alternate
   scalar/vector.  Output partials are bf16, host all-reduces in f32.
"""
import math
import numpy as np

S = 2048
D = 2048
HD = 128
H = 16
KV = 4
NCORES = 8
NT = S // 128          # 16 sequence tiles
DTC = D // 128         # 16 feature chunks
QH = H // NCORES       # 2 q-heads per core
ROPE_BASE = 10000.0
SCALE = 1.0 / math.sqrt(HD)
NEG = -1.0e9

_CACHE = {}


def _pt_layout(c):
    """Column layout of the P^T chunk-c buffer: per k-tile j the slice
    (offset, width) covering q-columns [max(512c, 128j), 512c+512)."""
    offs = []
    off = 0
    for j in range(4 * c + 4):
        w = min(512, 512 * c + 512 - 128 * j)
        offs.append((off, w))
        off += w
    return offs, off


def _rope(nc, pool, dst, src_ps, cos_sb, sin_sb, cols, BF16, ALU):
    """dst[:, cols] = src*cos + rotate_half(src)*sin  (src: psum [128, w])."""
    w = cols.stop - cols.start
    raw = pool.tile([128, w], BF16, tag="roperaw")
    rot = pool.tile([128, w], BF16, tag="roperot")
    t1 = pool.tile([128, w], BF16, tag="ropet1")
    nc.scalar.copy(out=raw, in_=src_ps)
    nc.vector.tensor_copy(out=rot[0:64, :], in_=raw[64:128, :])
    nc.vector.tensor_copy(out=rot[64:128, :], in_=raw[0:64, :])
    nc.vector.tensor_tensor(out=t1, in0=raw, in1=cos_sb[:, cols], op=ALU.mult)
    nc.vector.tensor_tensor(out=rot, in0=rot, in1=sin_sb[:, cols], op=ALU.mult)
    nc.vector.tensor_tensor(out=dst[:, cols], in0=t1, in1=rot, op=ALU.add)


def build_nc():
    import concourse.bacc as bacc
    import concourse.tile as tile
    from concourse import mybir

    F32 = mybir.dt.float32
    BF16 = mybir.dt.bfloat16
    AF = mybir.ActivationFunctionType
    ALU = mybir.AluOpType

    nc = bacc.Bacc("TRN2", target_bir_lowering=False, debug=False)
    hs_d = nc.dram_tensor("hs", [128, DTC * S], BF16, kind="ExternalInput").ap()
    wq_d = nc.dram_tensor("wq", [128, DTC * QH * 128], BF16, kind="ExternalInput").ap()
    wk_d = nc.dram_tensor("wk", [128, DTC * 128], BF16, kind="ExternalInput").ap()
    wv_d = nc.dram_tensor("wv", [128, DTC * 128], BF16, kind="ExternalInput").ap()
    wo_d = nc.dram_tensor("wo", [128, QH * D], BF16, kind="ExternalInput").ap()
    cos_d = nc.dram_tensor("cos", [128, S], BF16, kind="ExternalInput").ap()
    sin_d = nc.dram_tensor("sin", [128, S], BF16, kind="ExternalInput").ap()
    tri_d = nc.dram_tensor("tri", [128, 128], F32, kind="ExternalInput").ap()
    out_d = nc.dram_tensor("out", [128, NT * D], BF16, kind="ExternalOutput").ap()

    hs3 = hs_d.rearrange("p (t s) -> p t s", t=DTC)
    out3 = out_d.rearrange("p (t d) -> p t d", t=NT)

    HALF = S // 2
    QTR = S // 4

    with tile.TileContext(nc) as tc:
        with tc.tile_pool(name="consts", bufs=1) as consts, \
             tc.tile_pool(name="persist", bufs=1) as persist, \
             tc.tile_pool(name="hsp", bufs=8) as hsp, \
             tc.tile_pool(name="ropet", bufs=1) as ropet, \
             tc.tile_pool(name="lbp", bufs=2) as lbp, \
             tc.tile_pool(name="osb", bufs=4) as osb, \
             tc.tile_pool(name="qkvps", bufs=1, space="PSUM") as qkvps, \
             tc.tile_pool(name="sps", bufs=2, space="PSUM") as sps, \
             tc.tile_pool(name="lps", bufs=1, space="PSUM") as lpsp, \
             tc.tile_pool(name="pvps", bufs=1, space="PSUM") as pvps, \
             tc.tile_pool(name="pops", bufs=2, space="PSUM") as pops:
            tri_sb = consts.tile([128, 128], F32)
            ones_sb = consts.tile([128, 128], BF16)
            cos_sb = consts.tile([128, S], BF16)
            sin_sb = consts.tile([128, S], BF16)
            wq_sb = consts.tile([128, DTC, QH * 128], BF16)
            wk_sb = consts.tile([128, DTC, 128], BF16)
            wv_sb = consts.tile([128, DTC, 128], BF16)
            wo_sb = consts.tile([128, QH, D], BF16)

            qrot = [persist.tile([128, S], BF16, tag=f"qrot{h}", name=f"qrot{h}") for h in range(QH)]
            krot = persist.tile([128, S], BF16, tag="krot")
            vbf = persist.tile([128, S], BF16, tag="vbf")
            vnat = persist.tile([128, NT * 128], BF16, tag="vnat")
            vnat3 = vnat.rearrange("p (t f) -> p t f", t=NT)
            aout = [persist.tile([128, S], BF16, tag=f"aout{h}", name=f"aout{h}") for h in range(QH)]
            # P^T chunk buffers: pt[h][c] holds exp(s^T) for q-chunk c,
            # k-tiles j=0..4c+3 consecutively (see _pt_layout).
            pt_offs = {}
            pt = [[None] * 4 for _ in range(QH)]
            for h in range(QH):
                for c in range(4):
                    offs, L = _pt_layout(c)
                    pt_offs[c] = offs
                    pt[h][c] = persist.tile([128, L], BF16, tag=f"pt{h}_{c}",
                                            name=f"pt{h}_{c}")

            nc.vector.memset(ones_sb, 1.0)

            hst_tiles = {}

            def load_hst(sh, j):
                t = hsp.tile([128, 2, HALF], BF16, tag="hst", name=f"hst{sh}_{j}")
                nc.sync.dma_start(
                    out=t, in_=hs3[:, 2 * j:2 * j + 2, sh * HALF:(sh + 1) * HALF])
                hst_tiles[(sh, j)] = t
                return t

            # DMA prologue (all on the sync ring): first weight chunks + hs
            # tiles ahead of the bulk so the PE starts within ~3us.
            wq3 = wq_d.rearrange("p (t m) -> p t m", t=DTC)
            wk3 = wk_d.rearrange("p (t m) -> p t m", t=DTC)
            wv3 = wv_d.rearrange("p (t m) -> p t m", t=DTC)

            def load_w(a, b):
                nc.sync.dma_start(out=wq_sb[:, a:b, :], in_=wq3[:, a:b, :])
                nc.sync.dma_start(out=wk_sb[:, a:b, :], in_=wk3[:, a:b, :])
                nc.sync.dma_start(out=wv_sb[:, a:b, :], in_=wv3[:, a:b, :])

            nc.sync.dma_start(out=tri_sb, in_=tri_d)
            load_w(0, 2)
            load_hst(0, 0)
            load_hst(0, 1)
            load_w(2, 8)
            load_hst(0, 2)
            load_hst(0, 3)
            load_w(8, 16)
            for j in range(4, 8):
                load_hst(0, j)
            nc.sync.dma_start(out=cos_sb, in_=cos_d)
            nc.sync.dma_start(out=sin_sb, in_=sin_d)
            nc.sync.dma_start(out=wo_sb, in_=wo_d.rearrange("p (h m) -> p h m", h=QH))

            # ---------------- generators --------------------------------
            def qkv_gen(qtr):
                """Quarter qtr of the QKV projection in two 2-bank passes."""
                sh, qq = divmod(qtr, 2)
                cols = slice(qtr * QTR, (qtr + 1) * QTR)
                if qtr == 1:
                    for j in range(8):   # prefetch half 1 as slots free up
                        load_hst(1, j)
                # pass A: the two q heads
                pqa = [qkvps.tile([128, QTR], F32, tag=f"qk{m}", name=f"pqa{qtr}_{m}")
                       for m in range(QH)]
                for j in range(DTC // 2):
                    hst = hst_tiles[(sh, j)]
                    for t2 in range(2):
                        dt = 2 * j + t2
                        for m in range(QH):
                            nc.tensor.matmul(pqa[m], wq_sb[:, dt, m * 128:(m + 1) * 128],
                                             hst[:, t2, qq * QTR:(qq + 1) * QTR],
                                             start=(dt == 0), stop=(dt == DTC - 1))
                    yield
                for m in range(QH):
                    _rope(nc, ropet, qrot[m], pqa[m], cos_sb, sin_sb, cols, BF16, ALU)
                yield
                # pass B: k and v (reuses the two banks after rope A reads)
                pk = qkvps.tile([128, QTR], F32, tag="qk0", name=f"pk{qtr}")
                pv = qkvps.tile([128, QTR], F32, tag="qk1", name=f"pv{qtr}")
                for j in range(DTC // 2):
                    hst = hst_tiles[(sh, j)]
                    for t2 in range(2):
                        dt = 2 * j + t2
                        st, sp = dt == 0, dt == DTC - 1
                        nc.tensor.matmul(pk, wk_sb[:, dt, :],
                                         hst[:, t2, qq * QTR:(qq + 1) * QTR],
                                         start=st, stop=sp)
                        nc.tensor.matmul(pv, wv_sb[:, dt, :],
                                         hst[:, t2, qq * QTR:(qq + 1) * QTR],
                                         start=st, stop=sp)
                    yield
                _rope(nc, ropet, krot, pk, cos_sb, sin_sb, cols, BF16, ALU)
                nc.scalar.copy(out=vbf[:, cols], in_=pv)
                nc.sync.dma_start_transpose(
                    out=vnat3[:, 4 * qtr:4 * qtr + 4, :], in_=vbf[:, cols])
                yield

            def oproj_tile(t, g):
                o_sb = osb.tile([128, D], BF16, tag="osb")
                for n in range(D // 512):
                    po = pops.tile([128, 512], F32, tag="po", name=f"po{t}_{n}")
                    for hh in range(QH):
                        nc.tensor.matmul(po, aout[hh][:, t * 128:(t + 1) * 128],
                                         wo_sb[:, hh, n * 512:(n + 1) * 512],
                                         start=(hh == 0), stop=(hh == QH - 1))
                    on_scalar = (n % 2 == 0) if g == 3 else (n == 0)
                    if on_scalar:
                        nc.scalar.copy(out=o_sb[:, n * 512:(n + 1) * 512], in_=po)
                    else:
                        nc.vector.tensor_copy(out=o_sb[:, n * 512:(n + 1) * 512], in_=po)
                nc.sync.dma_start(out=out3[:, t, :], in_=o_sb)

            def attn_chain(c):
                """Scores^T + exp for q-chunk c, l/PV accumulation for units
                g=c (both heads), then o_proj group c.  Software-pipelined
                per k-tile j: the l/PV matmuls for j trail the score matmul
                for j+1 so the in-order PE queue rarely waits on an exp."""
                offs = pt_offs[c]
                jmax = 4 * c + 3
                for h in range(QH):
                    l_ps = lpsp.tile([128, 512], F32, tag="l", name=f"l{c}_{h}")
                    pv_ps = pvps.tile([128, 512], F32, tag="pv", name=f"pv{c}_{h}")

                    def lpv(j):
                        off, w = offs[j]
                        co = 512 - w
                        mv = pt[h][c][:, off:off + w]
                        nc.tensor.matmul(l_ps[:, co:512], ones_sb, mv,
                                         start=(j == 0), stop=(j == jmax))
                        nc.tensor.matmul(pv_ps[:, co:512],
                                         vnat[:, j * 128:(j + 1) * 128], mv,
                                         start=(j == 0), stop=(j == jmax))

                    prev = None
                    for j in range(jmax + 1):
                        off, w = offs[j]
                        qlo = max(512 * c, 128 * j)
                        s_ch = sps.tile([128, 512], F32, tag="s")
                        nc.tensor.matmul(s_ch[:, 0:w],
                                         krot[:, j * 128:(j + 1) * 128],
                                         qrot[h][:, qlo:qlo + w],
                                         start=True, stop=True)
                        if j >= 4 * c:   # diagonal block: first 128 cols
                            nc.vector.tensor_tensor(
                                out=s_ch[:, 0:128], in0=s_ch[:, 0:128],
                                in1=tri_sb, op=ALU.add)
                        if prev is not None:
                            lpv(prev)
                        nc.scalar.activation(out=pt[h][c][:, off:off + w],
                                             in_=s_ch[:, 0:w],
                                             func=AF.Exp, scale=SCALE)
                        yield
                        prev = j
                    lpv(jmax)
                    linv = lbp.tile([128, 512], F32, tag="linv", name=f"linv{c}_{h}")
                    nc.vector.reciprocal(out=linv, in_=l_ps)
                    nc.vector.tensor_tensor(
                        out=aout[h][:, c * 512:(c + 1) * 512],
                        in0=pv_ps, in1=linv, op=ALU.mult)
                    yield
                for t in range(4 * c, 4 * c + 4):
                    oproj_tile(t, c)
                    yield

            def run_full(gen):
                for _ in gen:
                    pass

            # master schedule
            run_full(qkv_gen(0))
            for q in range(1, 4):
                ga, gb = qkv_gen(q), attn_chain(q - 1)
                na = 18                       # qkv yields per quarter
                nb = 2 * (4 * (q - 1) + 5) + 4  # attn yields for chunk q-1
                ia = ib = 0
                da = db = False
                while not (da and db):
                    if not da and (db or ia * nb <= ib * na):
                        try:
                            next(ga)
                            ia += 1
                        except StopIteration:
                            da = True
                    else:
                        try:
                            next(gb)
                            ib += 1
                        except StopIteration:
                            db = True
            run_full(attn_chain(3))

    nc.compile()
    return nc


def _pm(x):
    """[n*128, M] row-major -> partition-major [128, n*M]."""
    n = x.shape[0] // 128
    return np.ascontiguousarray(
        x.reshape(n, 128, x.shape[1]).transpose(1, 0, 2).reshape(128, -1))


def prep_in_maps(hidden_states, position_ids, Wq, Wk, Wv, Wo):
    import ml_dtypes
    BF = ml_dtypes.bfloat16
    hs = np.asarray(hidden_states, dtype=np.float32).reshape(S, D)
    hsT_pm = _pm(np.ascontiguousarray(hs.T)).astype(BF)             # [128, DTC*S]

    pos = np.asarray(position_ids).reshape(S).astype(np.float32)
    inv = (ROPE_BASE ** (-np.arange(0, HD, 2, dtype=np.float32) / HD))  # [64]
    ang = np.concatenate([pos[None, :] * inv[:, None]] * 2, axis=0)     # [128, S]
    cos_t = np.cos(ang).astype(BF)
    sin_t = np.sin(ang).astype(np.float32)
    sin_signed = np.concatenate([-sin_t[:64], sin_t[64:]], axis=0).astype(BF)

    # transposed diagonal mask: triT[k, q] = 0 where q >= k else NEG
    q_idx = np.arange(128)[None, :]
    k_idx = np.arange(128)[:, None]
    triT = np.where(q_idx >= k_idx, 0.0, NEG).astype(np.float32)

    Wq = np.asarray(Wq, np.float32)
    Wk = np.asarray(Wk, np.float32)
    Wv = np.asarray(Wv, np.float32)
    Wo = np.asarray(Wo, np.float32)

    in_maps = []
    for c in range(NCORES):
        g = (c * QH) // (H // KV)          # kv head owned by this core
        wq_c = Wq[c * QH * 128:(c + 1) * QH * 128]      # [256, D]
        wk_c = Wk[g * 128:(g + 1) * 128]                # [128, D]
        wv_c = Wv[g * 128:(g + 1) * 128]                # [128, D]
        wo_c = Wo[:, c * QH * 128:(c + 1) * QH * 128]   # [D, 256]
        in_maps.append({
            "hs": hsT_pm,
            "wq": _pm(np.ascontiguousarray(wq_c.T)).astype(BF),
            "wk": _pm(np.ascontiguousarray(wk_c.T)).astype(BF),
            "wv": _pm(np.ascontiguousarray(wv_c.T)).astype(BF),
            "wo": _pm(np.ascontiguousarray(wo_c.T)).astype(BF),
            "cos": cos_t,
            "sin": sin_signed,
            "tri": triT,
        })
    return in_maps


def combine_outputs(results):
    total = np.zeros((S, D), np.float32)
    for r in results:
        o = np.asarray(r["out"], np.float32)
        total += o.reshape(128, NT, D).transpose(1, 0, 2).reshape(S, D)
    return total[None]


def kernel(hidden_states, attention_mask, position_ids, Wq, Wk, Wv, Wo):
    from concourse import bass_utils
    if "nc" not in _CACHE:
        _CACHE["nc"] = build_nc()
    nc = _CACHE["nc"]
    in_maps = prep_in_maps(hidden_states, position_ids, Wq, Wk, Wv, Wo)
    res = bass_utils.run_bass_kernel_spmd(nc, in_maps, core_ids=list(range(NCORES)))
    return combine_outputs(res.results)


# revision 25
# speedup vs baseline: 1.0552x; 1.0470x over previous
"""LlamaAttention (B=1, S=2048, D=2048, H=16, KV=4) on 8 TRN2 NeuronCores.

Tensor-parallel over heads: core c owns q-heads [2c, 2c+1] and kv-head c//2.
Each core computes partial = attn_out_c @ Wo[:, c-slice].T over the full
sequence; the all-reduce after o_proj happens on the host (sum of partials).

Layout strategy: everything on-chip lives feature-on-partitions ("transposed"):
  hsT [d, s], qT/kT/vT [hd, s], attn_outT [hd, s].  The host pre-transposes
hidden_states and weights into partition-major [128, N] bf16 arrays so every
DMA is contiguous; rope tables (bf16 cos / sign-adjusted sin) and the causal
diagonal mask block (transposed) are precomputed on host.

Key design point vs a classic flash-style schedule: attention scores are
computed TRANSPOSED (s^T[k, q] via stationary = krot k-tile, moving = qrot
q-columns), so the exp writes P^T [k-on-partitions] straight into SBUF and
P@V consumes it directly as the moving operand.  No DMA xbar transposes of P
are needed (each DMA transpose acts as a full DMA-subsystem barrier on TRN2,
which serialized the previous design).  The softmax row sums l[q] are
accumulated with an all-ones stationary matmul into a PSUM tile alongside
the P@V accumulation (every output partition holds the same l row), so the
aout drain is one reciprocal + one multiply, with no cross-partition
broadcast.

Schedule (all matmuls bf16):
 - QKV projects per quarter in TWO 2-bank passes (q-heads, then k/v), so
   PSUM fits: 2 qkv + 2 scores + 1 l + 1 pv + 2 o_proj = 8 banks.
 - attention chunk c (scores c for all k-tiles j<=4c+3, l+PV for units
   g=c, o_proj group c) interleaves into quarter c+1's projection stream;
   chunk 3 forms the tail as a per-j software pipeline
   [score MM j+1 | exp j | l/PV MM j] so the PE never waits a full exp.
 - softmax: plain exp (no running max: scores are O(6) sigma so fp32 exp
   cannot overflow); P^T stays unnormalized bf16, aout = pv * (1/l).
 - PSUM drains: rope uses one scalar cast then all-bf16 DVE ops (the
   rotate-half is a partition-shifted DVE copy); o_proj casts alternate
   scalar/vector.  Output partials are bf16, host all-reduces in f32.
"""
import math
import numpy as np

S = 2048
D = 2048
HD = 128
H = 16
KV = 4
NCORES = 8
NT = S // 128          # 16 sequence tiles
DTC = D // 128         # 16 feature chunks
QH = H // NCORES       # 2 q-heads per core
ROPE_BASE = 10000.0
SCALE = 1.0 / math.sqrt(HD)
NEG = -1.0e9

_CACHE = {}


def _pt_layout(c):
    """Column layout of the P^T chunk-c buffer: per k-tile j the slice
    (offset, width) covering q-columns [max(512c, 128j), 512c+512)."""
    offs = []
    off = 0
    for j in range(4 * c + 4):
        w = min(512, 512 * c + 512 - 128 * j)
        offs.append((off, w))
        off += w
    return offs, off


def _rope(nc, pool, dst, src_ps, cos_sb, sin_sb, cols, BF16, ALU):
    """dst[:, cols] = src*cos + rotate_half(src)*sin  (src: psum [128, w])."""
    w = cols.stop - cols.start
    raw = pool.tile([128, w], BF16, tag="roperaw")
    rot = pool.tile([128, w], BF16, tag="roperot")
    t1 = pool.tile([128, w], BF16, tag="ropet1")
    nc.scalar.copy(out=raw, in_=src_ps)
    nc.vector.tensor_copy(out=rot[0:64, :], in_=raw[64:128, :])
    nc.vector.tensor_copy(out=rot[64:128, :], in_=raw[0:64, :])
    nc.vector.tensor_tensor(out=t1, in0=raw, in1=cos_sb[:, cols], op=ALU.mult)
    nc.vector.tensor_tensor(out=rot, in0=rot, in1=sin_sb[:, cols], op=ALU.mult)
    nc.vector.tensor_tensor(out=dst[:, cols], in0=t1, in1=rot, op=ALU.add)


def build_nc():
    import concourse.bacc as bacc
    import concourse.tile as tile
    from concourse import mybir

    F32 = mybir.dt.float32
    BF16 = mybir.dt.bfloat16
    AF = mybir.ActivationFunctionType
    ALU = mybir.AluOpType

    nc = bacc.Bacc("TRN2", target_bir_lowering=False, debug=False)
    hs_d = nc.dram_tensor("hs", [128, DTC * S], BF16, kind="ExternalInput").ap()
    wq_d = nc.dram_tensor("wq", [128, DTC * QH * 128], BF16, kind="ExternalInput").ap()
    wk_d = nc.dram_tensor("wk", [128, DTC * 128], BF16, kind="ExternalInput").ap()
    wv_d = nc.dram_tensor("wv", [128, DTC * 128], BF16, kind="ExternalInput").ap()
    wo_d = nc.dram_tensor("wo", [128, QH * D], BF16, kind="ExternalInput").ap()
    cos_d = nc.dram_tensor("cos", [128, S], BF16, kind="ExternalInput").ap()
    sin_d = nc.dram_tensor("sin", [128, S], BF16, kind="ExternalInput").ap()
    tri_d = nc.dram_tensor("tri", [128, 128], F32, kind="ExternalInput").ap()
    out_d = nc.dram_tensor("out", [128, NT * D], BF16, kind="ExternalOutput").ap()

    hs3 = hs_d.rearrange("p (t s) -> p t s", t=DTC)
    out3 = out_d.rearrange("p (t d) -> p t d", t=NT)

    HALF = S // 2
    QTR = S // 4

    with tile.TileContext(nc) as tc:
        with tc.tile_pool(name="consts", bufs=1) as consts, \
             tc.tile_pool(name="persist", bufs=1) as persist, \
             tc.tile_pool(name="hsp", bufs=8) as hsp, \
             tc.tile_pool(name="ropet", bufs=1) as ropet, \
             tc.tile_pool(name="lbp", bufs=2) as lbp, \
             tc.tile_pool(name="osb", bufs=4) as osb, \
             tc.tile_pool(name="qkvps", bufs=1, space="PSUM") as qkvps, \
             tc.tile_pool(name="sps", bufs=2, space="PSUM") as sps, \
             tc.tile_pool(name="lps", bufs=1, space="PSUM") as lpsp, \
             tc.tile_pool(name="pvps", bufs=1, space="PSUM") as pvps, \
             tc.tile_pool(name="pops", bufs=2, space="PSUM") as pops:
            tri_sb = consts.tile([128, 128], F32)
            ones_sb = consts.tile([128, 128], BF16)
            cos_sb = consts.tile([128, S], BF16)
            sin_sb = consts.tile([128, S], BF16)
            wq_sb = consts.tile([128, DTC, QH * 128], BF16)
            wk_sb = consts.tile([128, DTC, 128], BF16)
            wv_sb = consts.tile([128, DTC, 128], BF16)
            wo_sb = consts.tile([128, QH, D], BF16)

            qrot = [persist.tile([128, S], BF16, tag=f"qrot{h}", name=f"qrot{h}") for h in range(QH)]
            krot = persist.tile([128, S], BF16, tag="krot")
            vbf = persist.tile([128, S], BF16, tag="vbf")
            vnat = persist.tile([128, NT * 128], BF16, tag="vnat")
            vnat3 = vnat.rearrange("p (t f) -> p t f", t=NT)
            aout = [persist.tile([128, S], BF16, tag=f"aout{h}", name=f"aout{h}") for h in range(QH)]
            # P^T chunk buffers: pt[h][c] holds exp(s^T) for q-chunk c,
            # k-tiles j=0..4c+3 consecutively (see _pt_layout).
            pt_offs = {}
            pt = [[None] * 4 for _ in range(QH)]
            for h in range(QH):
                for c in range(4):
                    offs, L = _pt_layout(c)
                    pt_offs[c] = offs
                    pt[h][c] = persist.tile([128, L], BF16, tag=f"pt{h}_{c}",
                                            name=f"pt{h}_{c}")

            nc.vector.memset(ones_sb, 1.0)

            hst_tiles = {}

            def load_hst(sh, j):
                t = hsp.tile([128, 2, HALF], BF16, tag="hst", name=f"hst{sh}_{j}")
                nc.sync.dma_start(
                    out=t, in_=hs3[:, 2 * j:2 * j + 2, sh * HALF:(sh + 1) * HALF])
                hst_tiles[(sh, j)] = t
                return t

            # DMA prologue (all on the sync ring): first weight chunks + hs
            # tiles ahead of the bulk so the PE starts within ~3us.
            wq3 = wq_d.rearrange("p (t m) -> p t m", t=DTC)
            wk3 = wk_d.rearrange("p (t m) -> p t m", t=DTC)
            wv3 = wv_d.rearrange("p (t m) -> p t m", t=DTC)

            # pass A consumes only wq, so stream all of wq first, then wk/wv
            # (needed ~8us later by pass B), interleaved with hs tiles.
            nc.sync.dma_start(out=tri_sb, in_=tri_d)
            nc.sync.dma_start(out=wq_sb[:, 0:4, :], in_=wq3[:, 0:4, :])
            load_hst(0, 0)
            load_hst(0, 1)
            nc.sync.dma_start(out=wq_sb[:, 4:10, :], in_=wq3[:, 4:10, :])
            load_hst(0, 2)
            nc.sync.dma_start(out=wq_sb[:, 10:16, :], in_=wq3[:, 10:16, :])
            load_hst(0, 3)
            nc.sync.dma_start(out=wk_sb, in_=wk3)
            nc.sync.dma_start(out=wv_sb, in_=wv3)
            for j in range(4, 8):
                load_hst(0, j)
            nc.sync.dma_start(out=cos_sb, in_=cos_d)
            nc.sync.dma_start(out=sin_sb, in_=sin_d)
            nc.sync.dma_start(out=wo_sb, in_=wo_d.rearrange("p (h m) -> p h m", h=QH))

            # ---------------- generators --------------------------------
            def qkv_gen(qtr):
                """Quarter qtr of the QKV projection in two 2-bank passes."""
                sh, qq = divmod(qtr, 2)
                cols = slice(qtr * QTR, (qtr + 1) * QTR)
                if qtr == 1:
                    for j in range(8):   # prefetch half 1 as slots free up
                        load_hst(1, j)
                # pass A: the two q heads
                pqa = [qkvps.tile([128, QTR], F32, tag=f"qk{m}", name=f"pqa{qtr}_{m}")
                       for m in range(QH)]
                for j in range(DTC // 2):
                    hst = hst_tiles[(sh, j)]
                    for t2 in range(2):
                        dt = 2 * j + t2
                        for m in range(QH):
                            nc.tensor.matmul(pqa[m], wq_sb[:, dt, m * 128:(m + 1) * 128],
                                             hst[:, t2, qq * QTR:(qq + 1) * QTR],
                                             start=(dt == 0), stop=(dt == DTC - 1))
                    yield
                for m in range(QH):
                    _rope(nc, ropet, qrot[m], pqa[m], cos_sb, sin_sb, cols, BF16, ALU)
                yield
                # pass B: k and v (reuses the two banks after rope A reads)
                pk = qkvps.tile([128, QTR], F32, tag="qk0", name=f"pk{qtr}")
                pv = qkvps.tile([128, QTR], F32, tag="qk1", name=f"pv{qtr}")
                for j in range(DTC // 2):
                    hst = hst_tiles[(sh, j)]
                    for t2 in range(2):
                        dt = 2 * j + t2
                        st, sp = dt == 0, dt == DTC - 1
                        nc.tensor.matmul(pk, wk_sb[:, dt, :],
                                         hst[:, t2, qq * QTR:(qq + 1) * QTR],
                                         start=st, stop=sp)
                        nc.tensor.matmul(pv, wv_sb[:, dt, :],
                                         hst[:, t2, qq * QTR:(qq + 1) * QTR],
                                         start=st, stop=sp)
                    yield
                _rope(nc, ropet, krot, pk, cos_sb, sin_sb, cols, BF16, ALU)
                nc.scalar.copy(out=vbf[:, cols], in_=pv)
                nc.sync.dma_start_transpose(
                    out=vnat3[:, 4 * qtr:4 * qtr + 4, :], in_=vbf[:, cols])
                yield

            def oproj_tile(t, g):
                o_sb = osb.tile([128, D], BF16, tag="osb")
                for n in range(D // 512):
                    po = pops.tile([128, 512], F32, tag="po", name=f"po{t}_{n}")
                    for hh in range(QH):
                        nc.tensor.matmul(po, aout[hh][:, t * 128:(t + 1) * 128],
                                         wo_sb[:, hh, n * 512:(n + 1) * 512],
                                         start=(hh == 0), stop=(hh == QH - 1))
                    on_scalar = (n % 2 == 0) if g == 3 else (n == 0)
                    if on_scalar:
                        nc.scalar.copy(out=o_sb[:, n * 512:(n + 1) * 512], in_=po)
                    else:
                        nc.vector.tensor_copy(out=o_sb[:, n * 512:(n + 1) * 512], in_=po)
                nc.sync.dma_start(out=out3[:, t, :], in_=o_sb)

            def attn_chain(c):
                """Scores^T + exp for q-chunk c, l/PV accumulation for units
                g=c (both heads), then o_proj group c.  Software-pipelined
                per k-tile j: the l/PV matmuls for j trail the score matmul
                for j+1 so the in-order PE queue rarely waits on an exp."""
                offs = pt_offs[c]
                jmax = 4 * c + 3
                for h in range(QH):
                    l_ps = lpsp.tile([128, 512], F32, tag="l", name=f"l{c}_{h}")
                    pv_ps = pvps.tile([128, 512], F32, tag="pv", name=f"pv{c}_{h}")

                    def lpv(j):
                        off, w = offs[j]
                        co = 512 - w
                        mv = pt[h][c][:, off:off + w]
                        nc.tensor.matmul(l_ps[:, co:512], ones_sb, mv,
                                         start=(j == 0), stop=(j == jmax))
                        nc.tensor.matmul(pv_ps[:, co:512],
                                         vnat[:, j * 128:(j + 1) * 128], mv,
                                         start=(j == 0), stop=(j == jmax))

                    LAG = 2
                    for j in range(jmax + 1):
                        off, w = offs[j]
                        qlo = max(512 * c, 128 * j)
                        s_ch = sps.tile([128, 512], F32, tag="s")
                        nc.tensor.matmul(s_ch[:, 0:w],
                                         krot[:, j * 128:(j + 1) * 128],
                                         qrot[h][:, qlo:qlo + w],
                                         start=True, stop=True)
                        if j >= 4 * c:   # diagonal block: first 128 cols
                            nc.vector.tensor_tensor(
                                out=s_ch[:, 0:128], in0=s_ch[:, 0:128],
                                in1=tri_sb, op=ALU.add)
                        if j >= LAG:
                            lpv(j - LAG)
                        nc.scalar.activation(out=pt[h][c][:, off:off + w],
                                             in_=s_ch[:, 0:w],
                                             func=AF.Exp, scale=SCALE)
                        yield
                    for j in range(max(0, jmax + 1 - LAG), jmax + 1):
                        lpv(j)
                    # 1/l = exp(-ln l): two scalar-engine ops in the broadcast
                    # orientation (a DVE reciprocal would cost ~6.5ns/column)
                    lnl = lbp.tile([128, 512], F32, tag="lnl", name=f"lnl{c}_{h}")
                    linv = lbp.tile([128, 512], F32, tag="linv", name=f"linv{c}_{h}")
                    nc.scalar.activation(out=lnl, in_=l_ps, func=AF.Ln)
                    nc.scalar.activation(out=linv, in_=lnl, func=AF.Exp, scale=-1.0)
                    nc.vector.tensor_tensor(
                        out=aout[h][:, c * 512:(c + 1) * 512],
                        in0=pv_ps, in1=linv, op=ALU.mult)
                    yield
                for t in range(4 * c, 4 * c + 4):
                    oproj_tile(t, c)
                    yield

            def run_full(gen):
                for _ in gen:
                    pass

            # master schedule
            run_full(qkv_gen(0))
            for q in range(1, 4):
                ga, gb = qkv_gen(q), attn_chain(q - 1)
                na = 18                       # qkv yields per quarter
                nb = 2 * (4 * (q - 1) + 5) + 4  # attn yields for chunk q-1
                ia = ib = 0
                da = db = False
                while not (da and db):
                    if not da and (db or ia * nb <= ib * na):
                        try:
                            next(ga)
                            ia += 1
                        except StopIteration:
                            da = True
                    else:
                        try:
                            next(gb)
                            ib += 1
                        except StopIteration:
                            db = True
            run_full(attn_chain(3))

    nc.compile()
    return nc


def _pm(x):
    """[n*128, M] row-major -> partition-major [128, n*M]."""
    n = x.shape[0] // 128
    return np.ascontiguousarray(
        x.reshape(n, 128, x.shape[1]).transpose(1, 0, 2).reshape(128, -1))


def prep_in_maps(hidden_states, position_ids, Wq, Wk, Wv, Wo):
    import ml_dtypes
    BF = ml_dtypes.bfloat16
    hs = np.asarray(hidden_states, dtype=np.float32).reshape(S, D)
    hsT_pm = _pm(np.ascontiguousarray(hs.T)).astype(BF)             # [128, DTC*S]

    pos = np.asarray(position_ids).reshape(S).astype(np.float32)
    inv = (ROPE_BASE ** (-np.arange(0, HD, 2, dtype=np.float32) / HD))  # [64]
    ang = np.concatenate([pos[None, :] * inv[:, None]] * 2, axis=0)     # [128, S]
    cos_t = np.cos(ang).astype(BF)
    sin_t = np.sin(ang).astype(np.float32)
    sin_signed = np.concatenate([-sin_t[:64], sin_t[64:]], axis=0).astype(BF)

    # transposed diagonal mask: triT[k, q] = 0 where q >= k else NEG
    q_idx = np.arange(128)[None, :]
    k_idx = np.arange(128)[:, None]
    triT = np.where(q_idx >= k_idx, 0.0, NEG).astype(np.float32)

    Wq = np.asarray(Wq, np.float32)
    Wk = np.asarray(Wk, np.float32)
    Wv = np.asarray(Wv, np.float32)
    Wo = np.asarray(Wo, np.float32)

    in_maps = []
    for c in range(NCORES):
        g = (c * QH) // (H // KV)          # kv head owned by this core
        wq_c = Wq[c * QH * 128:(c + 1) * QH * 128]      # [256, D]
        wk_c = Wk[g * 128:(g + 1) * 128]                # [128, D]
        wv_c = Wv[g * 128:(g + 1) * 128]                # [128, D]
        wo_c = Wo[:, c * QH * 128:(c + 1) * QH * 128]   # [D, 256]
        in_maps.append({
            "hs": hsT_pm,
            "wq": _pm(np.ascontiguousarray(wq_c.T)).astype(BF),
            "wk": _pm(np.ascontiguousarray(wk_c.T)).astype(BF),
            "wv": _pm(np.ascontiguousarray(wv_c.T)).astype(BF),
            "wo": _pm(np.ascontiguousarray(wo_c.T)).astype(BF),
            "cos": cos_t,
            "sin": sin_signed,
            "tri": triT,
        })
    return in_maps


def combine_outputs(results):
    total = np.zeros((S, D), np.float32)
    for r in results:
        o = np.asarray(r["out"], np.float32)
        total += o.reshape(128, NT, D).transpose(1, 0, 2).reshape(S, D)
    return total[None]


def kernel(hidden_states, attention_mask, position_ids, Wq, Wk, Wv, Wo):
    from concourse import bass_utils
    if "nc" not in _CACHE:
        _CACHE["nc"] = build_nc()
    nc = _CACHE["nc"]
    in_maps = prep_in_maps(hidden_states, position_ids, Wq, Wk, Wv, Wo)
    res = bass_utils.run_bass_kernel_spmd(nc, in_maps, core_ids=list(range(NCORES)))
    return combine_outputs(res.results)
